# revision 46
# baseline (speedup 1.0000x reference)
"""XNOR-Net BasicBlock (BN-sign-conv x2 + residual, training-mode BN) on 8 TRN2 cores.

Strategy (data-parallel on batch, 4 images/core), 223.9us timeline-sim:
  phase0: x loaded ONCE as fp16 (gpsimd DMA casts in flight, ~18us), x kept
          resident for prep1 + residual. Per-channel sum/sumsq split across
          DVE (4x fp16 tensor_scalar+accum) and ACT (Square+accum)
          -> AllGather (BN1 stats).
  conv  : s = sign(in - t) as fp8 +-1 on ACT into a row-interleaved padded
          plane (row r of group g at r*116+g*58) so each matmul's byte
          interval spans only the 8 rows it reads -> sign chunks unblock
          bands as they land. 3x3 conv = 9 DoubleRow fp8 matmuls per
          [128, 8x56] psum tile (integer-exact, zero PE idle). Epilogue:
          u = prelu(y) evicted by ACT (fused Prelu) for 10-11 tiles/image or
          DVE (ts copy + stt prelu) for the rest; one DVE bn_stats per tile
          (mean/M2); bn_aggr -> (m, v+m^2) per channel -> AllGather.
          Warm-up junk matmuls gated on the stats reduce keep the PE p-state
          hot through each AllReduce window.
  phase3: out = prelu(K*u2 + D + x, a3) in 17 half/quarter-image chunks:
          th=K*u2+D on DVE (4x ts), +x on DVE tt (2x) or a Pool side
          pipeline (3 chunks, prelu deferred), prelu on ACT, fp16 out DMA
          (host casts back to fp32).

Weights are binarized/laid out on host (tiny); all heavy tensors flow on device.
BN thresholds fold into per-channel sign biases: t1 = m1 - (b1/g1)*std1;
theta2 in u-units needs no sf factors since sign(prelu(y)-t) thresholds
commute; BN3 folds to K = g3/sqrt(v+eps'), D = b3 - K*m (eps' = eps/sf2^2).
"""

import sys

sys.path.insert(0, "/opt/trn_rl_repo")

import numpy as np

import concourse.bacc as bacc
import concourse.mybir as mybir
import concourse.tile as tile
from concourse.bass_utils import run_bass_kernel_spmd

F32 = mybir.dt.float32
F16 = mybir.dt.float16
F8 = mybir.dt.float8e4
F8NP = mybir.dt.np(F8)
F16NP = mybir.dt.np(F16)

AF = mybir.ActivationFunctionType
OP = mybir.AluOpType
DR = mybir.MatmulPerfMode.DoubleRow

NCORES = 8
B, C, H, W = 32, 256, 56, 56
BL = B // NCORES          # images per core
HW = H * W                # 3136
PW = W + 2                # 58 padded width
PLANE = PW * PW           # 3364 padded plane (58 rows x 58 cols)
ROWSTRIDE = 2 * PW        # 116: row-major plane, g0 row then g1 row
SPLANE = PW * ROWSTRIDE + 24   # 6752 bytes, rounded to %16
BAND = 8                  # output rows per matmul tile
NBAND = H // BAND         # 7
NFREE = BAND * W          # 448 psum free size (exact 56-wide windows)
NTOT = B * HW             # BN count (N*H*W over full batch)
EPS = 1e-5
OFFS = [(dh, dw) for dh in range(3) for dw in range(3)]
RH = H // 2               # row-half for split sign prep

# cvec column indices
CV_B1, CV_A1, CV_EPS1, CV_EPS2, CV_B2, CV_ISF1, CV_A2, CV_C2M, \
    CV_SF2N, CV_SF2SQN, CV_B3, CV_G3SF2, CV_G3, CV_B3V, CV_A3, CV_EPS = range(16)
CV_NCOLS = 16

_CACHE = {}


def _build():
    nc = bacc.Bacc(num_devices=NCORES)
    x_d = nc.declare_dram_parameter("x", [BL, C, H, W], F32, isOutput=False)
    w1_d = nc.declare_dram_parameter("w1s", [128, 2, 18 * 128], F8, isOutput=False)
    w2_d = nc.declare_dram_parameter("w2s", [128, 2, 18 * 128], F8, isOutput=False)
    cv_d = nc.declare_dram_parameter("cvec", [128, 2, CV_NCOLS], F32, isOutput=False)
    out_d = nc.declare_dram_parameter("out", [BL, C, H, W], F16, isOutput=True)

    # DRAM-side views: channel c -> (g = c // 128, p = c % 128)
    def x_view(n):
        return x_d[n].rearrange("(g p) h w -> p g (h w)", p=128)

    def out_view(n, g):
        return out_d[n].rearrange("(g p) h w -> p g h w", p=128)[:, g]

    with tile.TileContext(nc, num_cores=NCORES, pool_alloc_mode="queue") as tc:
        import contextlib

        es_u1 = contextlib.ExitStack()
        es_u2 = contextlib.ExitStack()
        es_ph0 = contextlib.ExitStack()
        with tc.tile_pool(name="consts", bufs=1) as cpool, \
                tc.tile_pool(name="weights", bufs=1) as wpool, \
                tc.tile_pool(name="spool", bufs=1) as spool, \
                tc.tile_pool(name="scr", bufs=3) as scrpool, \
                tc.tile_pool(name="psum", bufs=8, space="PSUM") as psum_pool, \
                tc.tile_pool(name="dram", bufs=1, space="DRAM") as dram_pool, \
                es_u2:

            # ---- persistent small tiles ----
            cvec = cpool.tile([128, 2, CV_NCOLS], F32, tag="cvec")
            st1 = cpool.tile([128, 16], F32, tag="st1")
            st2 = cpool.tile([128, 2, 28, 6], F32, tag="st2")
            st3 = cpool.tile([128, 2, 28, 6], F32, tag="st3")
            g1 = cpool.tile([128, 4], F32, tag="g1")
            g2 = cpool.tile([128, 4], F32, tag="g2")
            g3t = cpool.tile([128, 4], F32, tag="g3t")
            negt1 = cpool.tile([128, 2], F32, tag="negt1")
            negth2 = cpool.tile([128, 2], F32, tag="negth2")
            kvec = cpool.tile([128, 2], F32, tag="kvec")

            dvec = cpool.tile([128, 2], F32, tag="dvec")
            tmp_a = cpool.tile([128, 2], F32, tag="tmp_a")
            tmp_b = cpool.tile([128, 2], F32, tag="tmp_b")
            tmp_c = cpool.tile([128, 2], F32, tag="tmp_c")

            w1t = wpool.tile([128, 2, 18 * 128], F8, tag="w1t")
            w2t = wpool.tile([128, 2, 18 * 128], F8, tag="w2t")

            # pool open order fixes the release order (queue allocator):
            # u2 and x16 persist to the end; u1 freed after conv2 preps;
            # ph0 scratch freed after phase 0.
            u2_pool = es_u2.enter_context(tc.tile_pool(name="u2", bufs=BL))
            x16_pool = es_u2.enter_context(tc.tile_pool(name="x16", bufs=BL))
            u1_pool = es_u1.enter_context(tc.tile_pool(name="u1", bufs=BL))
            ph0_pool = es_ph0.enter_context(tc.tile_pool(name="ph0", bufs=1))

            s_tiles = [
                spool.tile([128, SPLANE], F8, tag="sa", name="sa"),
                spool.tile([128, SPLANE], F8, tag="sb", name="sb"),
            ]
            jmm = spool.tile([128, 2, NFREE], F8, tag="jmm", name="jmm")


            def s_plane(s, g):
                # row-interleaved layout: row r of group g at r*116 + g*58
                return s[:, 0:PW * ROWSTRIDE].rearrange(
                    "p (r g w) -> p r g w", g=2, w=PW)[:, :, g, :]

            def s_rhs(s, b, dh, dw):
                # moving operand for one matmul: rows b*8+dh..+7, both groups,
                # col window dw..dw+55 (exactly W outputs, no padding waste).
                # The byte interval only spans the 8 rows actually read, so
                # sign-prep chunks unblock bands as they land.
                r0 = b * BAND + dh
                sv = s[:, 0:PW * ROWSTRIDE].rearrange(
                    "p (r g w) -> p g r w", g=2, w=PW)
                return sv[:, :, r0:r0 + BAND, dw:dw + W]

            # =============== phase 0: load x as fp16, stats on DVE ===============
            # st1 col j = (k*2 + g)*4 + n   (k: 0=sum, 1=sumsq)
            x16 = []
            for n in range(BL):
                xt = x16_pool.tile([128, 2, HW], F16, tag="x16", name=f"x16_{n}")
                x16.append(xt)
            for n in range(BL):
                for g in range(2):
                    i = n * 2 + g
                    xg = x16[n][:, g, :]
                    nc.gpsimd.dma_start(xg, x_view(n)[:, g, :])
                    # sum(x): DVE tensor_scalar identity + accum (4x fp16)
                    pd = ph0_pool.tile([128, HW], F16, tag="ps", bufs=3)
                    nc.vector.tensor_scalar(
                        pd[:], xg, 1.0, 0.0, op0=OP.mult, op1=OP.add,
                        accum_out=st1[:, (0 * 2 + g) * 4 + n:(0 * 2 + g) * 4 + n + 1],
                    )
                    # sum(x^2): ACT Square+accum for 6 blocks, DVE tt+ts else
                    a2 = st1[:, (1 * 2 + g) * 4 + n:(1 * 2 + g) * 4 + n + 1]
                    if i in (0, 1, 2, 3, 4, 6):
                        pa = ph0_pool.tile([128, HW], F16, tag="ps", bufs=3)
                        nc.scalar.activation(pa[:], xg, AF.Square, accum_out=a2)
                    else:
                        pq = ph0_pool.tile([128, HW], F16, tag="ps", bufs=3)
                        pj = ph0_pool.tile([128, HW], F16, tag="ps", bufs=3)
                        nc.vector.tensor_tensor(pq[:], xg, xg, op=OP.mult)
                        nc.vector.tensor_scalar(
                            pj[:], pq[:], 1.0, 0.0, op0=OP.mult, op1=OP.add,
                            accum_out=a2,
                        )

            # weights/cvec on the SWDGE queue after the x16 gens so the x
            # transfers lead on the shared DMA device (w1t needed ~28us for
            # the warmup matmuls, cvec at theta1)
            nc.gpsimd.dma_start(w1t[:], w1_d[:])
            nc.gpsimd.dma_start(cvec[:], cv_d[:])
            nc.gpsimd.dma_start(w2t[:], w2_d[:])

            # s-plane borders must be zero before conv1; issued after the
            # x16 loads so the Pool SEQ doesn't delay them
            for s in s_tiles:
                nc.gpsimd.memset(s[:], 0.0)
            nc.gpsimd.memset(jmm[:], 0.0)

            def warmup(gate_src, wt, count, name):
                # keep the PE p-state hot through the AllReduce window: junk
                # matmuls gated on the stats reduce (DVE writes the gate) run
                # back-to-back until the real conv is ready
                nc.vector.tensor_scalar_mul(jmm[:, 0, 0:4], gate_src[:, 0:4], 0.0)
                jpt = psum_pool.tile([128, NFREE], F32, tag="pt", name=name)
                for _ in range(count):
                    nc.tensor.matmul(
                        jpt[:], wt[:, :, 0:128], jmm[:],
                        start=True, stop=True, perf_mode=DR,
                    )

            # reduce st1 [128, (k g) 4] -> r1 [128, 4], AllGather -> g1
            r1 = cpool.tile([128, 4], F32, tag="r1")
            nc.vector.reduce_sum(
                r1[:].rearrange("p (a b) -> p a b", b=1),
                st1[:].rearrange("p (kg t) -> p kg t", t=BL),
                axis=mybir.AxisListType.X,
            )
            warmup(r1, w1t, 195, "jpt1")
            ar1_i = dram_pool.tile([128, 4], F32, tag="ar1_i")
            ar1_o = dram_pool.tile([NCORES, 128, 4], F32, tag="ar1_o", addr_space="Shared")
            nc.sync.dma_start(ar1_i[:], r1[:])
            nc.gpsimd.collective_compute(
                "AllGather", OP.bypass, replica_groups=[list(range(NCORES))],
                ins=[ar1_i[:].opt()], outs=[ar1_o[:].opt()],
            )
            gth1 = cpool.tile([128, NCORES, 4], F32, tag="gth1")
            nc.sync.dma_start(gth1[:], ar1_o[:].rearrange("r p k -> p r k"))
            nc.vector.reduce_sum(
                g1[:].rearrange("p (a b) -> p a b", b=1),
                gth1[:].rearrange("p r k -> p k r"),
                axis=mybir.AxisListType.X,
            )
            es_ph0.close()  # phase-0 scratch fully consumed

            # ---- BN1 threshold: negt1 = B1*std1 - m1 ----
            g1v = g1[:].rearrange("p (k g) -> p k g", k=2)
            nc.vector.tensor_scalar_mul(tmp_a[:], g1v[:, 0], 1.0 / NTOT)      # m1
            nc.vector.tensor_scalar_mul(tmp_b[:], g1v[:, 1], 1.0 / NTOT)      # E[x^2]
            nc.vector.scalar_tensor_tensor(                                   # -m^2
                tmp_c[:], tmp_a[:], -1.0, tmp_a[:], op0=OP.mult, op1=OP.mult,
            )
            nc.vector.tensor_add(tmp_c[:], tmp_c[:], tmp_b[:])                # v1
            nc.scalar.activation(tmp_b[:], tmp_c[:], AF.Sqrt, bias=cvec[:, 0, CV_EPS:CV_EPS + 1], scale=1.0)  # std1
            nc.vector.tensor_mul(tmp_c[:], tmp_b[:], cvec[:, :, CV_B1])       # B1*std1
            nc.vector.tensor_sub(negt1[:], tmp_c[:], tmp_a[:])                # B1*std1 - m1

            # =============== conv pass helper ===============
            def conv_pass(widx, wt, stats, prep, u_pool):
                """One binary conv over all images.

                prep(n, s) emits the sign-writes of image n into s tile
                (4 ACT ops: 2 groups x 2 row-halves, so early bands can
                start as soon as the top half lands).
                stats col j = (k*2 + c)*28 + n*7 + b.
                Returns list of u tiles [128, 2, H, W] fp16 (prelu in y-units).
                """
                a_col = CV_A1 if widx == 0 else CV_A2
                u_tiles = []
                # image-0 prep in fine row-chunks right after the AR: the
                # first matmuls start after the top chunks of both groups
                for op in prep(0, s_tiles[0], fine=True):
                    op()
                for n in range(BL):
                    s = s_tiles[n % 2]
                    # next image's sign-prep quarters are interleaved between
                    # epilogue tiles so ACT serves this image's prelus first
                    pq = prep(n + 1, s_tiles[(n + 1) % 2]) if n + 1 < BL else []
                    ti = 0
                    ut = u_pool.tile([128, 2, H, W], F16, tag=f"u{widx}", name=f"u{widx}_{n}")
                    u_tiles.append(ut)
                    for cc in range(2):
                        for b in range(NBAND):
                            pt = psum_pool.tile(
                                [128, NFREE], F32, tag="pt",
                                name=f"pt{widx}_{n}_{cc}_{b}",
                            )
                            for o, (dh, dw) in enumerate(OFFS):
                                nc.tensor.matmul(
                                    pt[:],
                                    wt[:, :, (o * 2 + cc) * 128:(o * 2 + cc + 1) * 128],
                                    s_rhs(s, b, dh, dw),
                                    start=(o == 0), stop=(o == 8),
                                    perf_mode=DR,
                                )
                            pv = pt[:].rearrange("p (r w) -> p r w", w=W)
                            jt = n * NBAND + b
                            us = ut[:, cc, b * BAND:(b + 1) * BAND, :]
                            # u = prelu(y). PSUM eviction: ACT fused Prelu for
                            # 10-11 tiles per image, DVE copy + stt prelu for
                            # the rest. Stats via one DVE bn_stats per tile
                            # (count/mean/M2), aggregated post-conv.
                            if n == BL - 1 or cc == 0 or b < 3:
                                nc.scalar.activation(
                                    us, pv, AF.Prelu,
                                    alpha=cvec[:, cc, a_col:a_col + 1],
                                )
                            else:
                                yt = scrpool.tile([128, BAND, W], F16, tag="yt", bufs=3)
                                nc.vector.tensor_scalar_mul(yt[:], pv, 1.0)
                                nc.vector.scalar_tensor_tensor(
                                    us, yt[:], cvec[:, cc, a_col:a_col + 1], yt[:],
                                    op0=OP.mult, op1=OP.max,
                                )
                            nc.vector.bn_stats(
                                stats[:, cc, jt, :],
                                us.rearrange("p h w -> p (h w)"),
                            )
                            if ti in (1, 3, 5, 7, 9, 11, 12, 13) and pq:
                                pq.pop(0)()
                            ti += 1
                    while pq:
                        pq.pop(0)()
                return u_tiles

            # =============== conv1 ===============
            def mkprep(src_of, negt):
                def prep(n, s, fine=False):
                    hl = 14
                    ops = []
                    top = H if not fine else RH
                    for h0 in range(0, top, hl):
                        for g in range(2):
                            def op(h0=h0, g=g):
                                nc.scalar.activation(
                                    s_plane(s, g)[:, 1 + h0:1 + h0 + hl, 1:57],
                                    src_of(n, g)[:, h0:h0 + hl, :],
                                    AF.Sign, bias=negt[:, g:g + 1], scale=1.0,
                                )
                            ops.append(op)
                    if fine:
                        # post-AR prep is on the critical path: rows 28-55 go
                        # to the (idle) DVE as a 3-op ts sign chain so ACT and
                        # DVE prep the plane in parallel. sign(v) = max(min(
                        # v*1e30, 1), -1), exact +-1 in fp8 (0 stays 0).
                        for g in range(2):
                            def op(g=g):
                                sa = scrpool.tile([128, RH, W], F16, tag="sga", bufs=2)
                                sb = scrpool.tile([128, RH, W], F16, tag="sgb", bufs=2)
                                nc.vector.tensor_scalar(
                                    sa[:], src_of(n, g)[:, RH:H, :], 1.0,
                                    negt[:, g:g + 1], op0=OP.mult, op1=OP.add,
                                )
                                nc.vector.tensor_scalar(
                                    sb[:], sa[:], 1e30, 1.0,
                                    op0=OP.mult, op1=OP.min,
                                )
                                nc.vector.tensor_scalar(
                                    s_plane(s, g)[:, 1 + RH:1 + H, 1:57],
                                    sb[:], 1.0, -1.0, op0=OP.mult, op1=OP.max,
                                )
                            ops.append(op)
                    return ops
                return prep

            prep1 = mkprep(
                lambda n, g: x16[n][:, g, :].rearrange("p (h w) -> p h w", w=W),
                negt1,
            )

            u1 = conv_pass(0, w1t, st2, prep1, u1_pool)

            # aggregate st2 -> r2 = (m_c0, v_c0, m_c1, v_c1), AllGather -> g2
            r2 = cpool.tile([128, 4], F32, tag="r2")
            for cc in range(2):
                nc.vector.bn_aggr(r2[:, cc * 2:(cc + 1) * 2], st2[:, cc])
            # send (m, q=v+m^2): the cross-core combine then needs no squares
            r2v = r2[:].rearrange("p (c j) -> p j c", j=2)
            nc.vector.scalar_tensor_tensor(
                tmp_a[:], r2v[:, 0], 1.0, r2v[:, 0], op0=OP.mult, op1=OP.mult,
            )
            nc.vector.tensor_add(r2v[:, 1], r2v[:, 1], tmp_a[:])
            warmup(r2, w2t, 253, "jpt2")
            ar2_i = dram_pool.tile([128, 4], F32, tag="ar2_i")
            ar2_o = dram_pool.tile([NCORES, 128, 4], F32, tag="ar2_o", addr_space="Shared")
            nc.sync.dma_start(ar2_i[:], r2[:])
            nc.gpsimd.collective_compute(
                "AllGather", OP.bypass, replica_groups=[list(range(NCORES))],
                ins=[ar2_i[:].opt()], outs=[ar2_o[:].opt()],
            )
            gth2 = cpool.tile([128, NCORES, 4], F32, tag="gth2")
            nc.sync.dma_start(gth2[:], ar2_o[:].rearrange("r p k -> p r k"))

            def combine_mv(gth, m_out, v_out, red):
                """Full-batch per-channel (mean, var) from 8 cores' (m_i, q_i)
                where q_i = v_i + m_i^2: m = avg(m_i); v = avg(q_i) - m^2."""
                nc.vector.reduce_sum(
                    red[:], gth[:].rearrange("p r (c j) -> p j c r", j=2),
                    axis=mybir.AxisListType.X,
                )
                nc.vector.tensor_scalar_mul(m_out[:], red[:, 0], 1.0 / NCORES)
                nc.vector.tensor_scalar_mul(v_out[:], red[:, 1], 1.0 / NCORES)
                nc.vector.scalar_tensor_tensor(
                    tmp_c[:], m_out[:], -1.0, m_out[:], op0=OP.mult, op1=OP.mult,
                )
                nc.vector.tensor_add(v_out[:], v_out[:], tmp_c[:])

            # ---- BN2 threshold in u1 units: negth2 = B2*sigma_u - m_u ----
            red2 = cpool.tile([128, 2, 2], F32, tag="red2")
            m2u = cpool.tile([128, 2], F32, tag="m2u")
            v2u = cpool.tile([128, 2], F32, tag="v2u")
            combine_mv(gth2, m2u, v2u, red2)
            for g in range(2):
                nc.scalar.activation(
                    tmp_b[:, g:g + 1], v2u[:, g:g + 1], AF.Sqrt,
                    bias=cvec[:, g, CV_EPS1:CV_EPS1 + 1], scale=1.0,
                )
            nc.vector.tensor_mul(tmp_c[:], tmp_b[:], cvec[:, :, CV_B2])
            nc.vector.tensor_sub(negth2[:], tmp_c[:], m2u[:])

            # =============== conv2 ===============
            prep2 = mkprep(lambda n, g: u1[n][:, g, :, :], negth2)

            u2 = conv_pass(1, w2t, st3, prep2, u2_pool)

            # u1 fully consumed by prep2; release its pool so the queue
            # allocator can reuse the region for phase-3 tiles
            es_u1.close()

            # aggregate st3 -> r3, AllGather -> g3t
            r3 = cpool.tile([128, 4], F32, tag="r3")
            for cc in range(2):
                nc.vector.bn_aggr(r3[:, cc * 2:(cc + 1) * 2], st3[:, cc])
            r3v = r3[:].rearrange("p (c j) -> p j c", j=2)
            nc.vector.scalar_tensor_tensor(
                tmp_a[:], r3v[:, 0], 1.0, r3v[:, 0], op0=OP.mult, op1=OP.mult,
            )
            nc.vector.tensor_add(r3v[:, 1], r3v[:, 1], tmp_a[:])
            ar3_i = dram_pool.tile([128, 4], F32, tag="ar3_i")
            ar3_o = dram_pool.tile([NCORES, 128, 4], F32, tag="ar3_o", addr_space="Shared")
            nc.sync.dma_start(ar3_i[:], r3[:])
            nc.gpsimd.collective_compute(
                "AllGather", OP.bypass, replica_groups=[list(range(NCORES))],
                ins=[ar3_i[:].opt()], outs=[ar3_o[:].opt()],
            )
            gth3 = cpool.tile([128, NCORES, 4], F32, tag="gth3")
            nc.sync.dma_start(gth3[:], ar3_o[:].rearrange("r p k -> p r k"))

            # ---- BN3 affine in u2 units: K = g3/sqrt(v+eps'), D = b3 - K*m ----
            red3 = cpool.tile([128, 2, 2], F32, tag="red3")
            m3u = cpool.tile([128, 2], F32, tag="m3u")
            v3u = cpool.tile([128, 2], F32, tag="v3u")
            combine_mv(gth3, m3u, v3u, red3)
            for g in range(2):
                nc.scalar.activation(
                    tmp_b[:, g:g + 1], v3u[:, g:g + 1], AF.Sqrt,
                    bias=cvec[:, g, CV_EPS2:CV_EPS2 + 1], scale=1.0,
                )
            nc.vector.reciprocal(tmp_c[:], tmp_b[:])                          # 1/sqrt(v+eps')
            nc.vector.tensor_mul(kvec[:], tmp_c[:], cvec[:, :, CV_G3])        # K
            nc.vector.tensor_mul(tmp_a[:], kvec[:], m3u[:])                   # K*m
            nc.vector.tensor_sub(dvec[:], cvec[:, :, CV_B3V], tmp_a[:])       # D

            # =============== phase 3: out = prelu(K*u2 + D + x, a3) ===============
            # 16 half-image chunks for fine pipelining. th = K*u2+D on DVE
            # (fp16 4x ts). +x add: Pool tt for 5 chunks, DVE tt (2x) else.
            # prelu: ACT for 14 chunks, DVE ts+tt pair for 2. Out is fp16;
            # the host casts back to fp32.
            HB = H // 2
            POOL_W = {3, 7, 11}
            with tc.tile_pool(name="ph3", bufs=3) as p3pool:
                deferred = []
                i = 0
                QH = HB // 2
                for n in range(BL):
                    for g in range(2):
                        chunks = [(0, HB), (HB, HB)]
                        if n == BL - 1 and g == 1:
                            chunks = [(0, HB), (HB, QH), (HB + QH, QH)]
                        for h0, hl in chunks:
                            ug = u2[n][:, g, h0:h0 + hl, :]
                            xg = x16[n][:, g, :].rearrange(
                                "p (h w) -> p h w", w=W)[:, h0:h0 + hl, :]
                            a3c = cvec[:, g, CV_A3:CV_A3 + 1]
                            if i in POOL_W:
                                # th and +x fully on Pool (independent side
                                # pipeline); prelu deferred to DVE once the
                                # Pool add has drained
                                th = p3pool.tile([128, hl, W], F16, tag="thp",
                                                 name=f"th_{i}", bufs=2)
                                wt_ = p3pool.tile([128, hl, W], F16, tag="wtp",
                                                  name=f"wt_{i}", bufs=2)
                                nc.gpsimd.tensor_scalar(
                                    th[:], ug, kvec[:, g:g + 1], dvec[:, g:g + 1],
                                    op0=OP.mult, op1=OP.add,
                                )
                                nc.gpsimd.tensor_tensor(wt_[:], th[:], xg, op=OP.add)

                                def fin(wt_=wt_, a3c=a3c, n=n, g=g, h0=h0, hl=hl, i=i):
                                    ot = p3pool.tile([128, hl, W], F16, tag="ot",
                                                     name=f"ot_{i}", bufs=5)
                                    if i == 11:
                                        nc.scalar.activation(
                                            ot[:], wt_[:], AF.Prelu, alpha=a3c)
                                    else:
                                        aw = p3pool.tile([128, hl, W], F16, tag="aw",
                                                         name=f"aw_{i}", bufs=1)
                                        nc.vector.tensor_scalar_mul(aw[:], wt_[:], a3c)
                                        nc.vector.tensor_tensor(ot[:], aw[:], wt_[:], op=OP.max)
                                    nc.sync.dma_start(
                                        out_view(n, g)[:, h0:h0 + hl, :], ot[:])
                                deferred.append((i, fin))
                            else:
                                th = p3pool.tile([128, hl, W], F16, tag="th",
                                                 name=f"th_{i}", bufs=5)
                                wt_ = p3pool.tile([128, hl, W], F16, tag="wt",
                                                  name=f"wt_{i}", bufs=5)
                                ot = p3pool.tile([128, hl, W], F16, tag="ot",
                                                 name=f"ot_{i}", bufs=5)
                                nc.vector.tensor_scalar(
                                    th[:], ug, kvec[:, g:g + 1], dvec[:, g:g + 1],
                                    op0=OP.mult, op1=OP.add,
                                )
                                nc.vector.tensor_tensor(wt_[:], th[:], xg, op=OP.add)
                                if i >= 15:
                                    # tail chunks finish on DVE so ACT's
                                    # serial prelu queue isn't the drain
                                    aw = p3pool.tile([128, hl, W], F16, tag="awt",
                                                     name=f"awt_{i}", bufs=2)
                                    nc.vector.tensor_scalar_mul(aw[:], wt_[:], a3c)
                                    nc.vector.tensor_tensor(ot[:], aw[:], wt_[:], op=OP.max)
                                else:
                                    nc.scalar.activation(ot[:], wt_[:], AF.Prelu, alpha=a3c)
                                nc.sync.dma_start(
                                    out_view(n, g)[:, h0:h0 + hl, :], ot[:])
                            i += 1
                            if deferred and deferred[0][0] <= i - 5:
                                deferred.pop(0)[1]()
                for _, fin in deferred:
                    fin()

    nc.compile()
    return nc


def _host_prep(inputs):
    x = np.ascontiguousarray(np.asarray(inputs["x"], dtype=np.float32))
    w1 = np.asarray(inputs["w1"], dtype=np.float32)
    w2 = np.asarray(inputs["w2"], dtype=np.float32)

    def wprep(w):
        ws = np.sign(w).astype(np.float32)  # [co, ci, kh, kw]
        sf = np.abs(w).mean(axis=(1, 2, 3)).astype(np.float32)  # [256]
        arr = np.empty((128, 2, 18, 128), dtype=np.float32)
        for o, (dh, dw) in enumerate(OFFS):
            for cc in range(2):
                t = ws[cc * 128:(cc + 1) * 128, :, dh, dw]  # [m, ci]
                # arr[p, g, blk, m] = t[m, g*128 + p]
                arr[:, :, o * 2 + cc, :] = t.T.reshape(2, 128, 128).transpose(1, 0, 2)
        return arr.reshape(128, 2, 18 * 128).astype(F8NP), sf

    w1s, sf1 = wprep(w1)
    w2s, sf2 = wprep(w2)

    def vec(v):
        return np.asarray(v, dtype=np.float32).reshape(2, 128).T  # [p, g]

    g1v, b1v = inputs["g1"], inputs["b1"]
    g2v, b2v = inputs["g2"], inputs["b2"]
    g3v, b3v = inputs["g3"], inputs["b3"]
    a1, a2, a3 = inputs["a1"], inputs["a2"], inputs["a3"]

    cvec = np.zeros((128, 2, CV_NCOLS), dtype=np.float32)
    cvec[:, :, CV_B1] = vec(np.asarray(b1v) / np.asarray(g1v))
    cvec[:, :, CV_A1] = vec(np.asarray(a1))
    cvec[:, :, CV_EPS1] = vec(EPS / (sf1 * sf1))
    cvec[:, :, CV_EPS2] = vec(EPS / (sf2 * sf2))
    cvec[:, :, CV_B2] = vec(np.asarray(b2v) / np.asarray(g2v))
    cvec[:, :, CV_A2] = vec(np.asarray(a2))
    cvec[:, :, CV_G3] = vec(np.asarray(g3v))
    cvec[:, :, CV_B3V] = vec(np.asarray(b3v))
    cvec[:, :, CV_A3] = vec(np.asarray(a3))
    cvec[:, :, CV_EPS] = EPS

    return x, w1s, w2s, cvec


def run(inputs, trace=False):
    x, w1s, w2s, cvec = _host_prep(inputs)
    if "nc" not in _CACHE:
        _CACHE["nc"] = _build()
    nc = _CACHE["nc"]
    in_maps = [
        {"x": x[i * BL:(i + 1) * BL], "w1s": w1s, "w2s": w2s, "cvec": cvec}
        for i in range(NCORES)
    ]
    res = run_bass_kernel_spmd(nc, in_maps, list(range(NCORES)), trace=trace)
    out = np.concatenate([res.results[i]["out"] for i in range(NCORES)], axis=0)
    return out.astype(np.float32), res


def kernel(**inputs):
    out, _ = run(inputs, trace=False)
    return out


if __name__ == "__main__":
    # build-only check
    _build()
    print("BUILD OK")
